# revision 20
# baseline (speedup 1.0000x reference)
"""Trainium2 Bass kernel for a prototypical-network classification head.

Math (per task b):
    protos  = one_hot(labels).T @ support / counts          # (5, 1024)
    AB      = query @ protos.T                               # (75, 5)
    AA[q]   = |query[q]|^2 ;  BB[w] = |protos[w]|^2
    logits  = scale * (2*AB - AA - BB) / d                   # (75, 5)

Sharding: data-parallel over the 512 tasks across 8 NeuronCores (64 each).

Per-core dataflow (all f32):
  - query loaded as (120, 5*1024) tiles (8 tasks / supergroup, 40 tiles total)
    at ~full SBUF partition width; transposed on the PE via identity matmuls
    into qT (d on partitions).
  - protos computed with a block-diagonal one-hot as stationary operand
    (K=100 = 4 tasks' support rows), streamed against support in natural
    layout; transposed on PE into protosT.
  - AB and the query Gram matrix come from the same stationary qT slice;
    AA is the Gram diagonal, extracted with one fused tensor_tensor_reduce.
  - BB via ACT square-with-accumulate on protos, then a PE row transpose.
  - Final combine on DVE: logits = (2s/d)*AB - (s/d)*AA - (s/d)*BB.
"""

import math
import numpy as np
from contextlib import ExitStack

import concourse.bass as bass
import concourse.bacc as bacc
import concourse.tile as tile
from concourse import mybir
from concourse import bass_utils

F32 = mybir.dt.float32

# Problem shape (hardcoded per the task spec).
B, NQ, NS, D = 512, 75, 25, 1024
NW = 5
NCORES = 8
BPC = B // NCORES          # 64 tasks per core
DC = D // 128              # 8 contraction chunks

# Tiling
SG_TASKS = 8               # supergroup for query/AB (600 q-rows = 5 tiles of 120)
N_SG = BPC // SG_TASKS     # 8
QROWS_SG = SG_TASKS * NQ   # 600
QTILE = 120                # q-rows per transpose tile
KT = QROWS_SG // QTILE     # 5 q-tiles per supergroup
PG_TASKS = 16              # protos group
N_PG = BPC // PG_TASKS     # 4
SUB = 4                    # tasks per protos matmul (K = 4*25 = 100)

# If True use the PE transpose-mode instruction; else plain matmul-by-identity.
TRANSPOSE_MODE = True
# Build stages for debugging: 1=DMA only, 2=+qT transposes, 3=+protos,
# 4=+AB matmuls, 5=+AA extract, 6=+tensor_scalar, 7=full
STAGE = 7

_CACHE = {}


def _build(scale_val: float):
    s_d = scale_val / D
    nc = bacc.Bacc("TRN2", debug=False, target_bir_lowering=False, num_devices=NCORES)

    q_dram = nc.dram_tensor("q", [BPC * NQ, D], F32, kind="ExternalInput")
    sup_dram = nc.dram_tensor("sup", [BPC * NS, D], F32, kind="ExternalInput")
    oh_dram = nc.dram_tensor("oh4", [SUB * NS, BPC * NW], F32, kind="ExternalInput")
    id_dram = nc.dram_tensor("I128", [128, 128], F32, kind="ExternalInput")
    mask_dram = nc.dram_tensor("imask", [NQ, NQ], F32, kind="ExternalInput")
    out_dram = nc.dram_tensor("out", [BPC, NQ, NW], F32, kind="ExternalOutput")

    with tile.TileContext(nc) as tc, ExitStack() as ctx:
        singles = ctx.enter_context(tc.tile_pool(name="singles", bufs=1))
        qnat_pool = ctx.enter_context(tc.tile_pool(name="qnat", bufs=2))
        qtsg_pool = ctx.enter_context(tc.tile_pool(name="qtsg", bufs=2))
        sup_pool = ctx.enter_context(tc.tile_pool(name="sup", bufs=2))
        psb_pool = ctx.enter_context(tc.tile_pool(name="psb", bufs=2))
        ptsb_pool = ctx.enter_context(tc.tile_pool(name="ptsb", bufs=2))
        small_pool = ctx.enter_context(tc.tile_pool(name="small", bufs=2))
        scr_pool = ctx.enter_context(tc.tile_pool(name="scr", bufs=2))
        lg_pool = ctx.enter_context(tc.tile_pool(name="lg", bufs=2))

        qt_ps_pool = ctx.enter_context(tc.tile_pool(name="qtps", bufs=2, space="PSUM"))
        pp_ps_pool = ctx.enter_context(tc.tile_pool(name="ppps", bufs=3, space="PSUM"))
        ab_ps_pool = ctx.enter_context(tc.tile_pool(name="abps", bufs=3, space="PSUM"))

        oh_sb = singles.tile([SUB * NS, BPC * NW], F32)
        nc.sync.dma_start(out=oh_sb, in_=oh_dram.ap())
        id_sb = singles.tile([128, 128], F32)
        nc.sync.dma_start(out=id_sb, in_=id_dram.ap())
        mask_sb = singles.tile([NQ, NQ], F32)
        nc.sync.dma_start(out=mask_sb, in_=mask_dram.ap())
        ones75_sb = singles.tile([1, NQ], F32)
        nc.vector.memset(ones75_sb, 1.0)

        q_ap = q_dram.ap()       # (4800, 1024)
        sup_ap = sup_dram.ap()   # (1600, 1024)
        out_ap = out_dram.ap()   # (64, 75, 5)

        def do_transpose(out, in_, ident):
            if TRANSPOSE_MODE:
                nc.tensor.transpose(out, in_, ident)
            else:
                nc.tensor.matmul(out, in_, ident, start=True, stop=True)

        # protosT_sb per protos-group, kept alive across its 2 supergroups
        ptsb_tiles = {}

        def protos_group(pg):
            # --- load support for 16 tasks: rows [400*pg, 400*pg+400) ---
            sup_sb = sup_pool.tile([SUB * NS, SUB, D], F32, tag="sup")
            src = sup_ap[400 * pg:400 * (pg + 1), :].rearrange(
                "(j p) d -> p j d", j=SUB)
            nc.sync.dma_start(out=sup_sb, in_=src)
            if STAGE < 3:
                ptsb_tiles[pg] = (None, None)
                return

            # --- protos matmuls: per sub (4 tasks), per 512-col half ---
            protos_sb = psb_pool.tile([128, D], F32, tag="psb")
            bb_sp = small_pool.tile([128, 1], F32, tag="bbsp")
            bb_tmp = small_pool.tile([128, 1], F32, tag="bbtmp")

            for h in range(2):
                pp = pp_ps_pool.tile([128, 512], F32, tag="pp")
                # zero the junk rows so no stale/NaN bits ever feed the
                # full-width copy / BB accumulation below
                nc.vector.memset(pp, 0.0)
                for sub in range(SUB):
                    g4 = SUB * pg + sub
                    lhsT = oh_sb[:, 20 * g4:20 * (g4 + 1)]
                    rhs = sup_sb[:, sub, 512 * h:512 * (h + 1)]
                    outp = pp[32 * sub:32 * sub + 4 * NW, :]
                    nc.tensor.matmul(outp, lhsT, rhs, start=True, stop=True,
                                     tile_position=(0, 32 * sub))
                nc.scalar.copy(out=protos_sb[:, 512 * h:512 * (h + 1)], in_=pp)
                # BB partial: sum over this d-half of (sqrt(s/d)*p)^2
                scr = scr_pool.tile([128, 512], F32, tag="bbscr")
                acc = bb_sp if h == 0 else bb_tmp
                nc.scalar.activation(
                    out=scr, in_=pp,
                    func=mybir.ActivationFunctionType.Square,
                    scale=math.sqrt(s_d),
                    accum_out=acc)
            nc.vector.tensor_add(bb_sp, bb_sp, bb_tmp)

            # --- transpose protos -> protosT (8 PE blocks) ---
            ptsb = ptsb_pool.tile([128, D], F32, tag="ptsb")
            for hh in range(2):
                pt_ps = pp_ps_pool.tile([128, 512], F32, tag="pp")
                for cc in range(4):
                    c = 4 * hh + cc
                    do_transpose(pt_ps[:, 128 * cc:128 * (cc + 1)],
                                 protos_sb[:, 128 * c:128 * (c + 1)], id_sb)
                nc.scalar.copy(out=ptsb[:, 512 * hh:512 * (hh + 1)], in_=pt_ps)

            # --- BB broadcast: (128,1) -> (1,128) -> (75,128) via PE ---
            bb_row = ab_ps_pool.tile([1, 128], F32, tag="ab")
            nc.tensor.matmul(bb_row, bb_sp, id_sb, start=True, stop=True)
            bbrow_sb = small_pool.tile([1, 128], F32, tag="bbrow")
            nc.vector.tensor_copy(bbrow_sb, bb_row)
            bb_bc = ab_ps_pool.tile([NQ, 128], F32, tag="ab")
            nc.tensor.matmul(bb_bc, ones75_sb, bbrow_sb, start=True, stop=True)
            ptsb_tiles[pg] = (ptsb, bb_bc)

        def supergroup(sg):
            pg = sg // 2
            ptsb, bb_bc = ptsb_tiles[pg]

            # --- load 600 query rows as (120, 5, 1024) ---
            qnat = qnat_pool.tile([QTILE, KT, D], F32, tag="qnat")
            src = q_ap[QROWS_SG * sg:QROWS_SG * (sg + 1), :].rearrange(
                "(k p) d -> p k d", k=KT)
            nc.sync.dma_start(out=qnat, in_=src)

            # --- transpose to qT (d on partitions): (128, 8, 600) ---
            qt_sg = qtsg_pool.tile([128, DC, QROWS_SG], F32, tag="qtsg")
            if STAGE >= 2:
                for k in range(KT):
                    for hh in range(2):
                        qt_ps = qt_ps_pool.tile([128, 512], F32, tag="qtps")
                        for cc in range(4):
                            c = 4 * hh + cc
                            do_transpose(
                                qt_ps[:, 128 * cc:128 * cc + QTILE],
                                qnat[:, k, 128 * c:128 * (c + 1)],
                                id_sb[0:QTILE, 0:QTILE])
                        src_ap = qt_ps.rearrange("p (b x) -> p b x", b=4)[:, :, 0:QTILE]
                        dst_ap = qt_sg[:, 4 * hh:4 * hh + 4, QTILE * k:QTILE * (k + 1)]
                        nc.scalar.copy(out=dst_ap, in_=src_ap)

            # --- per-task AB + Gram, AA, combine ---
            lg = lg_pool.tile([NQ, SG_TASKS * NW], F32, tag="lg")
            if STAGE < 7:
                nc.vector.memset(lg, 0.0)
            for j in (range(SG_TASKS) if STAGE >= 4 else []):
                t = SG_TASKS * sg + j          # task id within core
                r16 = t % PG_TASKS
                pcol = 32 * (r16 // 4) + NW * (r16 % 4)   # protosT col offset
                ab = ab_ps_pool.tile([128, 80], F32, tag="ab")
                # MM1 (AB, cols 0:5) and MM2 (Gram, cols 5:80) share one psum
                # bank: start once (zeroes the bank), stop on the last matmul.
                for c in range(DC):
                    lhsT = qt_sg[:, c, NQ * j:NQ * (j + 1)]
                    nc.tensor.matmul(
                        ab[0:NQ, 0:NW],
                        lhsT, ptsb[:, 128 * c + pcol:128 * c + pcol + NW],
                        start=(c == 0), stop=False)
                    nc.tensor.matmul(
                        ab[0:NQ, NW:80],
                        lhsT, lhsT,
                        start=False, stop=(c == DC - 1))
                if STAGE < 5:
                    continue
                # AA (scaled by s/d) = diag of Gram; mask carries the s/d
                # factor (host-baked), so aas = (s/d) * AA after the reduce.
                aas = small_pool.tile([NQ, 1], F32, tag="aas")
                scr = scr_pool.tile([NQ, NQ], F32, tag="gramscr")
                nc.vector.tensor_tensor(
                    out=scr, in0=ab[0:NQ, NW:80], in1=mask_sb,
                    op=mybir.AluOpType.mult)
                nc.vector.tensor_reduce(
                    out=aas, in_=scr, axis=mybir.AxisListType.X,
                    op=mybir.AluOpType.add)
                if STAGE < 6:
                    continue
                # logits = AB*(2s/d) - AA_s - BB_s
                lgs = lg[:, NW * j:NW * (j + 1)]
                nc.vector.tensor_scalar(
                    out=lgs, in0=ab[0:NQ, 0:NW],
                    scalar1=2.0 * s_d, scalar2=aas,
                    op0=mybir.AluOpType.mult, op1=mybir.AluOpType.subtract)
                if STAGE < 7:
                    continue
                nc.vector.tensor_tensor(
                    out=lgs, in0=lgs,
                    in1=bb_bc[0:NQ, pcol:pcol + NW],
                    op=mybir.AluOpType.subtract)

            # --- store: (75, 8, 5) -> out[8sg:8sg+8, :, :] ---
            dst = out_ap[SG_TASKS * sg:SG_TASKS * (sg + 1), :, :].transpose([1, 0, 2])
            nc.sync.dma_start(out=dst, in_=lg.rearrange("q (j w) -> q j w", j=SG_TASKS))

        for pg in range(N_PG):
            protos_group(pg)
            supergroup(2 * pg)
            supergroup(2 * pg + 1)

    nc.compile()
    return nc


def _host_prep(query, support, labels, n_way, scale_val=1.0):
    """Build per-core input maps (numpy only, no FLOPs beyond tiny one-hot)."""
    q = np.ascontiguousarray(np.asarray(query, dtype=np.float32))
    sup = np.ascontiguousarray(np.asarray(support, dtype=np.float32))
    lab = np.asarray(labels).astype(np.int64)

    # one_hot / counts, exactly like the reference
    oh = (lab[:, :, None] == np.arange(n_way)[None, None, :]).astype(np.float32)
    counts = oh.sum(axis=1)  # (B, n_way)
    with np.errstate(divide="ignore", invalid="ignore"):
        ohs = oh / counts[:, None, :]  # (B, 25, 5)

    I128 = np.eye(128, dtype=np.float32)
    imask = np.eye(NQ, dtype=np.float32) * (scale_val / D)

    in_maps = []
    for c in range(NCORES):
        t0 = BPC * c
        oh4 = np.zeros((SUB * NS, BPC * NW), dtype=np.float32)
        for g4 in range(BPC // SUB):
            for i in range(SUB):
                oh4[NS * i:NS * (i + 1), 20 * g4 + NW * i:20 * g4 + NW * (i + 1)] = \
                    ohs[t0 + SUB * g4 + i]
        in_maps.append({
            "q": q[t0:t0 + BPC].reshape(BPC * NQ, D),
            "sup": sup[t0:t0 + BPC].reshape(BPC * NS, D),
            "oh4": oh4,
            "I128": I128,
            "imask": imask,
        })
    return in_maps


TRACE = False
last_exec_time_ns = None


def kernel(**inputs):
    global last_exec_time_ns
    query = inputs["query"]
    support = inputs["support"]
    labels = inputs["support_labels"]
    n_way = int(np.asarray(inputs.get("n_way", NW)))
    scale = float(np.asarray(inputs["scale"]).reshape(-1)[0])
    assert n_way == NW

    key = scale
    if key not in _CACHE:
        _CACHE[key] = _build(scale)
    nc = _CACHE[key]

    in_maps = _host_prep(query, support, labels, n_way, scale)
    res = bass_utils.run_bass_kernel_spmd(
        nc, in_maps, core_ids=list(range(NCORES)), trace=TRACE)
    last_exec_time_ns = res.exec_time_ns
    out = np.concatenate([res.results[c]["out"] for c in range(NCORES)], axis=0)
    return out.astype(np.float32)
